# revision 1
# baseline (speedup 1.0000x reference)
"""SAGEConv (mean aggregation) + ReLU on 8 Trainium2 NeuronCores.

    out = relu( (mean_{j in N(i)} x_j) @ W_l.T + b_l + x_i @ W_r.T )

Strategy (graph/data parallel, hardcoded for N=100000, E=1600000, D=128):
  - Destination nodes are partitioned across 8 cores (12500 nodes each).
  - Edges are grouped by (core, 128-dst block, src chunk); source features are
    fetched with the Q7 `dma_gather` custom instruction (int16 indices, so x is
    split into 4 chunks of 25000 rows).
  - Per dst block, a scaled one-hot selection matrix S[e, d] =
    (dstrel[e]==d) * (1/deg[dst]) is built on the vector engine with one
    tensor_scalar(is_equal, mult) op per 128-edge tile, and the segment mean
    msgT[f, d] = sum_e Xg[e, f] * S[e, d] accumulates in PSUM on the tensor
    engine.
  - out[d, :] = msgT.T @ W_l.T + x_loc.T.T @ W_r.T + b_l (three PSUM-accumulated
    matmuls, bias via a K=1 matmul with a ones row), then ReLU on the scalar
    engine.
  - Weights are replicated; x chunks are replicated; x_loc arrives
    pre-transposed per core so no on-device transpose is needed.
"""

import math

import numpy as np

import concourse.bass as bass
import concourse.bacc as bacc
import concourse.mybir as mybir
import concourse.tile as tile
from concourse import library_config
from concourse.bass_utils import run_bass_kernel_spmd

N = 100000
E = 1600000
D = 128
NCORES = 8
NPC = N // NCORES  # 12500 dst nodes per core
NB = 100  # 128-dst blocks per core (98 real + 2 pad)
G = 4  # blocks per superblock (gather granularity)
NSB = NB // G  # 25 superblocks
NQ = 4  # src chunks (int16 index limit)
CH = N // NQ  # 25000 rows per chunk
F32 = mybir.dt.float32
I16 = mybir.dt.int16


def _build_nc(t4, reps=1, nsb=NSB):
    slots_b = NQ * t4  # tiles per block
    slots_sb = G * slots_b  # slots per superblock buffer
    nidx_q = G * t4 * 128  # indices per (superblock, chunk) gather
    idxw_cols = NQ * nidx_q // 16

    nc = bacc.Bacc("TRN2", target_bir_lowering=False, debug=False)
    xq = [nc.dram_tensor(f"x{q}", [CH, D], F32, kind="ExternalInput") for q in range(NQ)]
    nb = nsb * G
    idxs = nc.dram_tensor("idxs", [nsb, 128, idxw_cols], I16, kind="ExternalInput")
    dstrel = nc.dram_tensor("dstrel", [nsb, 128, slots_sb], F32, kind="ExternalInput")
    redge = nc.dram_tensor("redge", [nsb, 128, slots_sb], F32, kind="ExternalInput")
    iota = nc.dram_tensor("iota", [128, 128], F32, kind="ExternalInput")
    xloct = nc.dram_tensor("xloct", [128, nb * 128], F32, kind="ExternalInput")
    wlt = nc.dram_tensor("wlt", [D, D], F32, kind="ExternalInput")
    wrt = nc.dram_tensor("wrt", [D, D], F32, kind="ExternalInput")
    misc = nc.dram_tensor("misc", [2, D], F32, kind="ExternalInput")
    out = nc.dram_tensor("out", [nb * 128, D], F32, kind="ExternalOutput")

    with tile.TileContext(nc) as tc:
        with tc.tile_critical():
            nc.gpsimd.load_library(library_config.mlp)
        with (
            tc.tile_pool(name="const", bufs=1) as cpool,
            tc.tile_pool(name="xg", bufs=2) as xgpool,
            tc.tile_pool(name="meta", bufs=2) as mpool,
            tc.tile_pool(name="s", bufs=6) as spool,
            tc.tile_pool(name="work", bufs=3) as wpool,
            tc.tile_pool(name="psum", bufs=2, space="PSUM") as ppool,
        ):
            iota_sb = cpool.tile([128, 128], F32)
            nc.sync.dma_start(out=iota_sb[:], in_=iota[:])
            wlt_sb = cpool.tile([D, D], F32)
            nc.sync.dma_start(out=wlt_sb[:], in_=wlt[:])
            wrt_sb = cpool.tile([D, D], F32)
            nc.sync.dma_start(out=wrt_sb[:], in_=wrt[:])
            blr_sb = cpool.tile([1, D], F32)
            nc.sync.dma_start(out=blr_sb[:], in_=misc[0:1, :])
            ones_sb = cpool.tile([1, D], F32)
            nc.sync.dma_start(out=ones_sb[:], in_=misc[1:2, :])

            def body():
                for sb in range(nsb):
                    idx_sb = mpool.tile([128, idxw_cols], I16, tag="idx")
                    nc.sync.dma_start(out=idx_sb[:], in_=idxs[sb])
                    dr_sb = mpool.tile([128, slots_sb], F32, tag="dr")
                    nc.sync.dma_start(out=dr_sb[:], in_=dstrel[sb])
                    re_sb = mpool.tile([128, slots_sb], F32, tag="re")
                    nc.sync.dma_start(out=re_sb[:], in_=redge[sb])

                    xg = xgpool.tile([128, slots_sb * 128], F32, tag="xg")
                    nslot_q = G * t4
                    # dma_gather is only reliable up to 1024 idxs per
                    # instruction (HW-verified: 1024 ok, 2048 faults), so
                    # split each chunk gather into <=8-slot pieces.
                    MAXS = 8
                    for q in range(NQ):
                        for s0 in range(0, nslot_q, MAXS):
                            ns = min(MAXS, nslot_q - s0)
                            base = q * nslot_q + s0
                            nidx = ns * 128
                            c0 = (q * nidx_q + s0 * 128) // 16
                            nc.gpsimd.dma_gather(
                                xg[:, base * 128 : (base + ns) * 128].rearrange(
                                    "p (s d) -> p s d", d=128
                                ),
                                xq[q][:],
                                idx_sb[:, c0 : c0 + nidx // 16],
                                nidx,
                                nidx,
                                D,
                            )

                    for bi in range(G):
                        b = sb * G + bi
                        msgt = ppool.tile([128, 128], F32, tag="msgt")
                        n_tiles = NQ * t4
                        for j in range(n_tiles):
                            q, t = divmod(j, t4)
                            sl = (q * G + bi) * t4 + t  # slot in xg
                            col = bi * slots_b + j  # column in dr/re
                            s_t = spool.tile([128, 128], F32, tag="s")
                            nc.vector.tensor_scalar(
                                out=s_t[:],
                                in0=iota_sb[:],
                                scalar1=dr_sb[:, col : col + 1],
                                scalar2=re_sb[:, col : col + 1],
                                op0=mybir.AluOpType.is_equal,
                                op1=mybir.AluOpType.mult,
                            )
                            nc.tensor.matmul(
                                out=msgt[:],
                                lhsT=xg[:, sl * 128 : (sl + 1) * 128],
                                rhs=s_t[:],
                                start=(j == 0),
                                stop=(j == n_tiles - 1),
                            )
                        aggt = wpool.tile([128, 128], F32, tag="aggt")
                        nc.vector.tensor_copy(out=aggt[:], in_=msgt[:])
                        xct = wpool.tile([128, 128], F32, tag="xct")
                        nc.sync.dma_start(out=xct[:], in_=xloct[:, b * 128 : (b + 1) * 128])
                        outp = ppool.tile([128, D], F32, tag="outp")
                        nc.tensor.matmul(out=outp[:], lhsT=aggt[:], rhs=wlt_sb[:], start=True, stop=False)
                        nc.tensor.matmul(out=outp[:], lhsT=xct[:], rhs=wrt_sb[:], start=False, stop=False)
                        nc.tensor.matmul(out=outp[:], lhsT=ones_sb[:], rhs=blr_sb[:], start=False, stop=True)
                        outs = wpool.tile([128, D], F32, tag="outs")
                        nc.scalar.activation(outs[:], outp[:], mybir.ActivationFunctionType.Relu)
                        nc.sync.dma_start(out=out[b * 128 : (b + 1) * 128, :], in_=outs[:])

            if reps == 1:
                body()
            else:
                with tc.For_i(0, reps, 1):
                    body()
    nc.compile()
    return nc


def _prep(x, edge_index):
    """Host-side sharding: group edges by (core, block, chunk), pad to tiles."""
    x = np.ascontiguousarray(np.asarray(x, dtype=np.float32))
    src = np.asarray(edge_index[0], dtype=np.int64)
    dst = np.asarray(edge_index[1], dtype=np.int64)

    deg = np.bincount(dst, minlength=N)
    rec = (1.0 / np.maximum(deg, 1.0)).astype(np.float32)

    c = dst // NPC
    local = dst - c * NPC
    b = local >> 7
    drel = (local & 127).astype(np.float32)
    q = src // CH
    i16 = (src - q * CH).astype(np.int16)
    re = rec[dst]

    key = ((c * NB + b) * NQ + q).astype(np.int64)
    order = np.argsort(key, kind="stable")
    key_s = key[order]
    counts = np.bincount(key, minlength=NCORES * NB * NQ)
    t4 = max(1, math.ceil(counts.max() / 128))
    cap = t4 * 128

    starts = np.zeros(NCORES * NB * NQ, np.int64)
    np.cumsum(counts[:-1], out=starts[1:])
    pos = np.arange(E, dtype=np.int64) - starts[key_s]
    dest = key_s * cap + pos

    total = NCORES * NB * NQ * cap
    idx_pad = np.zeros(total, np.int16)
    drel_pad = np.full(total, -1.0, np.float32)
    re_pad = np.zeros(total, np.float32)
    idx_pad[dest] = i16[order]
    drel_pad[dest] = drel[order]
    re_pad[dest] = re[order]

    idx_pad = idx_pad.reshape(NCORES, NSB, G, NQ, cap)
    drel_pad = drel_pad.reshape(NCORES, NSB, G, NQ, t4, 128)
    re_pad = re_pad.reshape(NCORES, NSB, G, NQ, t4, 128)

    # idxw[c]: [NSB, 128, NQ*G*cap/16]; per (sb,q) wrap G*cap idxs into 16
    # partitions (idx i -> [i%16, i//16]) and replicate to 128 partitions.
    flat = idx_pad.transpose(0, 1, 3, 2, 4).reshape(NCORES, NSB, NQ, G * cap)
    w = flat.reshape(NCORES, NSB, NQ, G * cap // 16, 16).transpose(0, 1, 2, 4, 3)
    w = np.tile(w, (1, 1, 1, 8, 1))  # [c, NSB, NQ, 128, G*cap/16]
    idxw = np.ascontiguousarray(
        w.transpose(0, 1, 3, 2, 4).reshape(NCORES, NSB, 128, NQ * G * cap // 16)
    )

    # dstrel/redge[c]: [NSB, 128, G*NQ*t4] with col = bi*(NQ*t4) + q*t4 + t
    dr_dev = np.ascontiguousarray(
        drel_pad.transpose(0, 1, 5, 2, 3, 4).reshape(NCORES, NSB, 128, G * NQ * t4)
    )
    re_dev = np.ascontiguousarray(
        re_pad.transpose(0, 1, 5, 2, 3, 4).reshape(NCORES, NSB, 128, G * NQ * t4)
    )

    xq_np = [x[qq * CH : (qq + 1) * CH] for qq in range(NQ)]

    xloct = np.zeros((NCORES, 128, NB * 128), np.float32)
    for cc in range(NCORES):
        xl = np.zeros((NB * 128, D), np.float32)
        xl[:NPC] = x[cc * NPC : (cc + 1) * NPC]
        xloct[cc] = xl.T

    return t4, xq_np, idxw, dr_dev, re_dev, xloct


def _in_maps(inputs):
    x = inputs["x"]
    edge_index = inputs["edge_index"]
    w_l = np.asarray(inputs["W_l"], dtype=np.float32)
    b_l = np.asarray(inputs["b_l"], dtype=np.float32)
    w_r = np.asarray(inputs["W_r"], dtype=np.float32)

    t4, xq_np, idxw, dr_dev, re_dev, xloct = _prep(x, edge_index)

    iota_np = np.ascontiguousarray(
        np.broadcast_to(np.arange(128, dtype=np.float32), (128, 128))
    )
    wlt_np = np.ascontiguousarray(w_l.T)
    wrt_np = np.ascontiguousarray(w_r.T)
    misc_np = np.stack([b_l, np.ones(D, np.float32)])

    in_maps = []
    for c in range(NCORES):
        m = {f"x{q}": xq_np[q] for q in range(NQ)}
        m.update(
            idxs=idxw[c], dstrel=dr_dev[c], redge=re_dev[c], iota=iota_np,
            xloct=xloct[c], wlt=wlt_np, wrt=wrt_np, misc=misc_np,
        )
        in_maps.append(m)
    return t4, in_maps


def _run(inputs, reps=1):
    t4, in_maps = _in_maps(inputs)
    nc = _build_nc(t4, reps=reps)
    res = run_bass_kernel_spmd(nc, in_maps, core_ids=list(range(NCORES)))
    out = np.concatenate(
        [res.results[c]["out"][:NPC] for c in range(NCORES)], axis=0
    )
    return out


def kernel(**inputs) -> np.ndarray:
    return _run(inputs, reps=1)



# revision 19
# speedup vs baseline: 1.0023x; 1.0023x over previous
"""SAGEConv (mean aggregation) + ReLU on 8 Trainium2 NeuronCores.

    out = relu( (mean_{j in N(i)} x_j) @ W_l.T + b_l + x_i @ W_r.T )

Strategy (graph/data parallel, hardcoded for N=100000, E=1600000, D=128):
  - Destination nodes are partitioned across 8 cores (12500 nodes each).
  - Edges are grouped by (core, 128-dst block, src chunk); source features are
    fetched with the Q7 `dma_gather` custom instruction (int16 indices, so x is
    split into 4 chunks of 25000 rows).
  - Per dst block, a scaled one-hot selection matrix S[e, d] =
    (dstrel[e]==d) * (1/deg[dst]) is built on the vector engine with one
    tensor_scalar(is_equal, mult) op per 128-edge tile, and the segment mean
    msgT[f, d] = sum_e Xg[e, f] * S[e, d] accumulates in PSUM on the tensor
    engine.
  - out[d, :] = msgT.T @ W_l.T + x_loc.T.T @ W_r.T + b_l (three PSUM-accumulated
    matmuls, bias via a K=1 matmul with a ones row), then ReLU on the scalar
    engine.
  - Weights are replicated; x chunks are replicated; x_loc arrives
    pre-transposed per core so no on-device transpose is needed.

Perf notes from profiling (HW, NTFF traces):
  - The kernel is GPSIMD-bound: Q7 descriptor generation for dma_gather costs
    ~8ns/idx + ~0.6us/instruction and runs at ~98% occupancy; DMA itself is
    <50% busy, tensor ~46%, vector ~77%. Total ~2.3ms.
  - Splitting gathers per (block, chunk) (400 x 640-idx instructions) to skip
    padding via trailing -1 idxs + per-count num_idxs_reg HANGS the device
    (ring bookkeeping?); with 0-padding it measures 2.78ms (worse: more
    per-instruction fixed cost). bf16 x (256B descriptors) does not reduce
    Q7 time and slightly slows the gather slope. Larger gathers (2048 idx)
    fault (ring capacity ~128 descs/engine). Multi-queue gathers
    (num_swdge_queues=4, queue_num=q) crash unrecoverably. Hence this shape:
    fp32, 300 gathers of <=1024 idxs, single queue.
"""

import math

import numpy as np

import concourse.bass as bass
import concourse.bacc as bacc
import concourse.mybir as mybir
import concourse.tile as tile
from concourse import library_config
from concourse.bass_utils import run_bass_kernel_spmd

N = 100000
E = 1600000
D = 128
NCORES = 8
NPC = N // NCORES  # 12500 dst nodes per core
NB = 100  # 128-dst blocks per core (98 real + 2 pad)
G = 4  # blocks per superblock (gather granularity)
NSB = NB // G  # 25 superblocks
NQ = 4  # src chunks (int16 index limit)
CH = N // NQ  # 25000 rows per chunk
F32 = mybir.dt.float32
I16 = mybir.dt.int16


def _build_nc(t4, reps=1, nsb=NSB):
    slots_b = NQ * t4  # tiles per block
    slots_sb = G * slots_b  # slots per superblock buffer
    nidx_q = G * t4 * 128  # indices per (superblock, chunk) gather
    idxw_cols = NQ * nidx_q // 16

    nc = bacc.Bacc("TRN2", target_bir_lowering=False, debug=False)
    xq = [nc.dram_tensor(f"x{q}", [CH, D], F32, kind="ExternalInput") for q in range(NQ)]
    nb = nsb * G
    idxs = nc.dram_tensor("idxs", [nsb, 128, idxw_cols], I16, kind="ExternalInput")
    dstrel = nc.dram_tensor("dstrel", [nsb, 128, slots_sb], F32, kind="ExternalInput")
    redge = nc.dram_tensor("redge", [nsb, 128, slots_sb], F32, kind="ExternalInput")
    iota = nc.dram_tensor("iota", [128, 128], F32, kind="ExternalInput")
    xloct = nc.dram_tensor("xloct", [128, nb * 128], F32, kind="ExternalInput")
    wlt = nc.dram_tensor("wlt", [D, D], F32, kind="ExternalInput")
    wrt = nc.dram_tensor("wrt", [D, D], F32, kind="ExternalInput")
    misc = nc.dram_tensor("misc", [2, D], F32, kind="ExternalInput")
    out = nc.dram_tensor("out", [nb * 128, D], F32, kind="ExternalOutput")

    with tile.TileContext(nc) as tc:
        with tc.tile_critical():
            nc.gpsimd.load_library(library_config.mlp)
        with (
            tc.tile_pool(name="const", bufs=1) as cpool,
            tc.tile_pool(name="xg", bufs=2) as xgpool,
            tc.tile_pool(name="meta", bufs=2) as mpool,
            tc.tile_pool(name="s", bufs=6) as spool,
            tc.tile_pool(name="work", bufs=3) as wpool,
            tc.tile_pool(name="psum", bufs=2, space="PSUM") as ppool,
        ):
            iota_sb = cpool.tile([128, 128], F32)
            nc.sync.dma_start(out=iota_sb[:], in_=iota[:])
            wlt_sb = cpool.tile([D, D], F32)
            nc.sync.dma_start(out=wlt_sb[:], in_=wlt[:])
            wrt_sb = cpool.tile([D, D], F32)
            nc.sync.dma_start(out=wrt_sb[:], in_=wrt[:])
            blr_sb = cpool.tile([1, D], F32)
            nc.sync.dma_start(out=blr_sb[:], in_=misc[0:1, :])
            ones_sb = cpool.tile([1, D], F32)
            nc.sync.dma_start(out=ones_sb[:], in_=misc[1:2, :])

            def body():
                for sb in range(nsb):
                    idx_sb = mpool.tile([128, idxw_cols], I16, tag="idx")
                    nc.sync.dma_start(out=idx_sb[:], in_=idxs[sb])
                    dr_sb = mpool.tile([128, slots_sb], F32, tag="dr")
                    nc.sync.dma_start(out=dr_sb[:], in_=dstrel[sb])
                    re_sb = mpool.tile([128, slots_sb], F32, tag="re")
                    nc.sync.dma_start(out=re_sb[:], in_=redge[sb])

                    xg = xgpool.tile([128, slots_sb * 128], F32, tag="xg")
                    nslot_q = G * t4
                    # dma_gather is only reliable up to 1024 idxs per
                    # instruction (HW-verified: 1024 ok, 2048 faults), so
                    # split each chunk gather into <=8-slot pieces.
                    MAXS = 8
                    for q in range(NQ):
                        for s0 in range(0, nslot_q, MAXS):
                            ns = min(MAXS, nslot_q - s0)
                            base = q * nslot_q + s0
                            nidx = ns * 128
                            c0 = (q * nidx_q + s0 * 128) // 16
                            nc.gpsimd.dma_gather(
                                xg[:, base * 128 : (base + ns) * 128].rearrange(
                                    "p (s d) -> p s d", d=128
                                ),
                                xq[q][:],
                                idx_sb[:, c0 : c0 + nidx // 16],
                                nidx,
                                nidx,
                                D,
                            )

                    for bi in range(G):
                        b = sb * G + bi
                        msgt = ppool.tile([128, 128], F32, tag="msgt")
                        n_tiles = NQ * t4
                        for j in range(n_tiles):
                            q, t = divmod(j, t4)
                            sl = (q * G + bi) * t4 + t  # slot in xg
                            col = bi * slots_b + j  # column in dr/re
                            s_t = spool.tile([128, 128], F32, tag="s")
                            nc.vector.tensor_scalar(
                                out=s_t[:],
                                in0=iota_sb[:],
                                scalar1=dr_sb[:, col : col + 1],
                                scalar2=re_sb[:, col : col + 1],
                                op0=mybir.AluOpType.is_equal,
                                op1=mybir.AluOpType.mult,
                            )
                            nc.tensor.matmul(
                                out=msgt[:],
                                lhsT=xg[:, sl * 128 : (sl + 1) * 128],
                                rhs=s_t[:],
                                start=(j == 0),
                                stop=(j == n_tiles - 1),
                            )
                        aggt = wpool.tile([128, 128], F32, tag="aggt")
                        nc.vector.tensor_copy(out=aggt[:], in_=msgt[:])
                        xct = wpool.tile([128, 128], F32, tag="xct")
                        nc.sync.dma_start(out=xct[:], in_=xloct[:, b * 128 : (b + 1) * 128])
                        outp = ppool.tile([128, D], F32, tag="outp")
                        nc.tensor.matmul(out=outp[:], lhsT=aggt[:], rhs=wlt_sb[:], start=True, stop=False)
                        nc.tensor.matmul(out=outp[:], lhsT=xct[:], rhs=wrt_sb[:], start=False, stop=False)
                        nc.tensor.matmul(out=outp[:], lhsT=ones_sb[:], rhs=blr_sb[:], start=False, stop=True)
                        outs = wpool.tile([128, D], F32, tag="outs")
                        nc.scalar.activation(outs[:], outp[:], mybir.ActivationFunctionType.Relu)
                        nc.sync.dma_start(out=out[b * 128 : (b + 1) * 128, :], in_=outs[:])

            if reps == 1:
                body()
            else:
                with tc.For_i(0, reps, 1):
                    body()
    nc.compile()
    return nc


def _prep(x, edge_index):
    """Host-side sharding: group edges by (core, block, chunk), pad to tiles."""
    x = np.ascontiguousarray(np.asarray(x, dtype=np.float32))
    src = np.asarray(edge_index[0], dtype=np.int64)
    dst = np.asarray(edge_index[1], dtype=np.int64)

    deg = np.bincount(dst, minlength=N)
    rec = (1.0 / np.maximum(deg, 1.0)).astype(np.float32)

    c = dst // NPC
    local = dst - c * NPC
    b = local >> 7
    drel = (local & 127).astype(np.float32)
    q = src // CH
    i16 = (src - q * CH).astype(np.int16)
    re = rec[dst]

    key = ((c * NB + b) * NQ + q).astype(np.int64)
    order = np.argsort(key, kind="stable")
    key_s = key[order]
    counts = np.bincount(key, minlength=NCORES * NB * NQ)
    t4 = max(1, math.ceil(counts.max() / 128))
    cap = t4 * 128

    starts = np.zeros(NCORES * NB * NQ, np.int64)
    np.cumsum(counts[:-1], out=starts[1:])
    pos = np.arange(E, dtype=np.int64) - starts[key_s]
    dest = key_s * cap + pos

    total = NCORES * NB * NQ * cap
    idx_pad = np.zeros(total, np.int16)
    drel_pad = np.full(total, -1.0, np.float32)
    re_pad = np.zeros(total, np.float32)
    idx_pad[dest] = i16[order]
    drel_pad[dest] = drel[order]
    re_pad[dest] = re[order]

    idx_pad = idx_pad.reshape(NCORES, NSB, G, NQ, cap)
    drel_pad = drel_pad.reshape(NCORES, NSB, G, NQ, t4, 128)
    re_pad = re_pad.reshape(NCORES, NSB, G, NQ, t4, 128)

    # idxw[c]: [NSB, 128, NQ*G*cap/16]; per (sb,q) wrap G*cap idxs into 16
    # partitions (idx i -> [i%16, i//16]) and replicate to 128 partitions.
    flat = idx_pad.transpose(0, 1, 3, 2, 4).reshape(NCORES, NSB, NQ, G * cap)
    w = flat.reshape(NCORES, NSB, NQ, G * cap // 16, 16).transpose(0, 1, 2, 4, 3)
    w = np.tile(w, (1, 1, 1, 8, 1))  # [c, NSB, NQ, 128, G*cap/16]
    idxw = np.ascontiguousarray(
        w.transpose(0, 1, 3, 2, 4).reshape(NCORES, NSB, 128, NQ * G * cap // 16)
    )

    # dstrel/redge[c]: [NSB, 128, G*NQ*t4] with col = bi*(NQ*t4) + q*t4 + t
    dr_dev = np.ascontiguousarray(
        drel_pad.transpose(0, 1, 5, 2, 3, 4).reshape(NCORES, NSB, 128, G * NQ * t4)
    )
    re_dev = np.ascontiguousarray(
        re_pad.transpose(0, 1, 5, 2, 3, 4).reshape(NCORES, NSB, 128, G * NQ * t4)
    )

    xq_np = [x[qq * CH : (qq + 1) * CH] for qq in range(NQ)]

    xloct = np.zeros((NCORES, 128, NB * 128), np.float32)
    for cc in range(NCORES):
        xl = np.zeros((NB * 128, D), np.float32)
        xl[:NPC] = x[cc * NPC : (cc + 1) * NPC]
        xloct[cc] = xl.T

    return t4, xq_np, idxw, dr_dev, re_dev, xloct


def _in_maps(inputs):
    x = inputs["x"]
    edge_index = inputs["edge_index"]
    w_l = np.asarray(inputs["W_l"], dtype=np.float32)
    b_l = np.asarray(inputs["b_l"], dtype=np.float32)
    w_r = np.asarray(inputs["W_r"], dtype=np.float32)

    t4, xq_np, idxw, dr_dev, re_dev, xloct = _prep(x, edge_index)

    iota_np = np.ascontiguousarray(
        np.broadcast_to(np.arange(128, dtype=np.float32), (128, 128))
    )
    wlt_np = np.ascontiguousarray(w_l.T)
    wrt_np = np.ascontiguousarray(w_r.T)
    misc_np = np.stack([b_l, np.ones(D, np.float32)])

    in_maps = []
    for c in range(NCORES):
        m = {f"x{q}": xq_np[q] for q in range(NQ)}
        m.update(
            idxs=idxw[c], dstrel=dr_dev[c], redge=re_dev[c], iota=iota_np,
            xloct=xloct[c], wlt=wlt_np, wrt=wrt_np, misc=misc_np,
        )
        in_maps.append(m)
    return t4, in_maps


def _run(inputs, reps=1):
    t4, in_maps = _in_maps(inputs)
    nc = _build_nc(t4, reps=reps)
    res = run_bass_kernel_spmd(nc, in_maps, core_ids=list(range(NCORES)))
    out = np.concatenate(
        [res.results[c]["out"][:NPC] for c in range(NCORES)], axis=0
    )
    return out


def kernel(**inputs) -> np.ndarray:
    return _run(inputs, reps=1)
